# revision 15
# baseline (speedup 1.0000x reference)
"""Trainium2 Bass kernel for a 2-layer GraphSAGE(mean) encoder (8 NeuronCores).

Sharding (per spec hint: dst-node partition + per-partition halo of src feats):
  - Layer 0: core c owns dst0 nodes {d : d % 8 == c} (6250 -> 49 tiles of 128
    rows).  Host prep stages, per core, the incoming-edge halo: for every
    edge a bf16 log1p(x[src]) row placed at its (chunk, partition) slot,
    laid out partition-major so the device streams it with dense,
    full-bandwidth DMAs.  Segment-sum runs on the PE via one-hot matmuls
    (M[e,d] = (dstloc[e]==d)*inv_cnt, built on the DVE in bf16).
  - Layer 1: edges are assigned to cores by src1 % 8 so every message row is
    core-local; h1 rows are fetched with dma_gather (int16 idx) in 3
    src-subranges so the gathers overlap the tail of layer 0; each core
    computes partial segment sums over all 10000 dst1 nodes (permuted
    layout grouped by dst1 % 8) and two ReduceScatter(add) halves deliver
    each core its own 1250 dst1 rows (second half overlaps head compute).
  - Weights are replicated (bf16); final projection / relu / L2-normalize /
    heads run on the owning core; the host interleaves per-core outputs.

kernel(**inputs) takes the FULL inputs (as produced by reference.setup_inputs)
and returns (z_loc, z_scale) as float32 numpy arrays of shape [10000, 32].
"""

import math

import numpy as np
import ml_dtypes

import concourse.bass as bass
import concourse.bacc as bacc
import concourse.mybir as mybir
from concourse.bass_utils import run_bass_kernel_spmd
from concourse.masks import make_identity
from concourse.tile import TileContext

# ----------------------------------------------------------------------------
# Problem constants (hardcoded; the harness always uses these shapes).
# ----------------------------------------------------------------------------
N0, N1, N2 = 200000, 50000, 10000
E0, E1 = 800000, 160000
F_IN, H, L = 128, 256, 32
NC = 8
P = 128

T0 = math.ceil(N1 // NC / P)  # 49 layer-0 dst tiles per core
R0 = T0 * P  # 6272 padded local rows per core
GTILES = 2  # L0 tiles per staged group
B1 = math.ceil(N2 // NC / P) * P  # 1280: padded per-core block of dst1 nodes
T1P = NC * B1 // P  # 80 partial tiles (10240 permuted rows)
T1 = B1 // P  # 10 final tiles per core
NSUB = 3  # layer-1 src subranges (for gather/L0 overlap)
SUBR = 2048  # h1 rows per subrange (last one has R0-2*2048=2176)
EPS_NORM = 1e-12
GCHUNKS = 8  # max chunks (128 rows) per dma_gather instruction

f32 = mybir.dt.float32
bf16 = mybir.dt.bfloat16
i16 = mybir.dt.int16
bfnp = ml_dtypes.bfloat16


def _ranks_from_sorted(keys_sorted):
    """rank of each element within its equal-key run (keys_sorted ascending)."""
    n = keys_sorted.shape[0]
    if n == 0:
        return np.zeros(0, np.int64)
    new_run = np.empty(n, bool)
    new_run[0] = True
    new_run[1:] = keys_sorted[1:] != keys_sorted[:-1]
    starts = np.flatnonzero(new_run)
    run_ids = np.cumsum(new_run) - 1
    return np.arange(n) - starts[run_ids]


def _chunk_layout(core, cell, n_cells):
    """Shared chunk layout: cap[cell] = ceil(max_core count / P) chunks.

    Returns (cap, chunk_start, total, e_chunk, e_part) where e_chunk/e_part
    give each edge's slot (chunks shared across cores, slots per core).
    """
    counts = np.zeros((NC, n_cells), np.int64)
    np.add.at(counts, (core, cell), 1)
    cap = np.ceil(counts.max(axis=0) / P).astype(np.int64)
    chunk_start = np.concatenate([[0], np.cumsum(cap)])
    total = int(chunk_start[-1])
    order = np.lexsort((cell, core))
    key = core.astype(np.int64) * n_cells + cell
    ranks = _ranks_from_sorted(key[order])
    e_chunk = np.empty(core.shape[0], np.int64)
    e_part = np.empty(core.shape[0], np.int64)
    e_chunk[order] = chunk_start[cell[order]] + ranks // P
    e_part[order] = ranks % P
    return cap, chunk_start, total, e_chunk, e_part


class _Prep:
    """Host-side graph preprocessing shared by all cores."""

    def __init__(self, x, src0, dst0, src1, dst1):
        src0 = np.asarray(src0).astype(np.int64)
        dst0 = np.asarray(dst0).astype(np.int64)
        src1 = np.asarray(src1).astype(np.int64)
        dst1 = np.asarray(dst1).astype(np.int64)

        deg0 = np.bincount(dst0, minlength=N1)
        inv0 = (1.0 / np.maximum(deg0, 1)).astype(np.float32)
        deg1 = np.bincount(dst1, minlength=N2)
        inv1 = (1.0 / np.maximum(deg1, 1)).astype(np.float32)

        xt = np.log1p(np.asarray(x, np.float32)).astype(bfnp)  # [N0, F_IN]

        # ---------------- layer 0: dst-tile partition + edge halo ----------
        c0 = dst0 % NC
        t0 = (dst0 // NC) // P
        loc0 = ((dst0 // NC) % P).astype(np.float32)
        w0 = inv0[dst0]
        cap0, cs0, self.l0_chunks, ech, ept = _chunk_layout(c0, t0, T0)
        self.l0_cap = cap0
        self.l0_chunk_start = cs0
        # halo [NC][128, chunks, F_IN] bf16; dstw [NC][128, chunks, 2] f32
        self.halo = np.zeros((NC, P, self.l0_chunks, F_IN), bfnp)
        self.l0_dstw = np.zeros((NC, P, self.l0_chunks, 2), np.float32)
        self.l0_dstw[..., 0] = 200.0  # pad dstloc -> one-hot of zeros
        self.halo[c0, ept, ech] = xt[src0]
        self.l0_dstw[c0, ept, ech, 0] = loc0
        self.l0_dstw[c0, ept, ech, 1] = w0

        # per-core self rows, transposed: xselfT [NC][F_IN, R0] bf16
        self.xselfT = np.zeros((NC, F_IN, R0), bfnp)
        for c in range(NC):
            rows = xt[c::NC][: N1 // NC]  # [6250, F_IN]
            self.xselfT[c, :, : rows.shape[0]] = rows.T

        # ---------------- layer 1: src-core partition, permuted dst -------
        c1 = src1 % NC
        srow = src1 // NC  # local h1 row on owning core
        sub = np.minimum(srow // SUBR, NSUB - 1)
        srel = srow - sub * SUBR  # row within subrange tensor
        pd = (dst1 % NC) * B1 + dst1 // NC
        t1 = pd // P
        loc1 = (pd % P).astype(np.float32)
        w1 = inv1[dst1]
        cell = sub * T1P + t1  # layout: subphase-major, then tile
        cap1, cs1, self.l1_chunks, ech1, ept1 = _chunk_layout(c1, cell, NSUB * T1P)
        self.l1_cap = cap1.reshape(NSUB, T1P)
        self.l1_chunk_start = cs1.reshape(-1)[: NSUB * T1P + 1]
        # subphase chunk spans
        self.sub_span = []  # (chunk0, nchunks) per subphase
        for s in range(NSUB):
            a = int(cs1[s * T1P])
            b = int(cs1[(s + 1) * T1P]) if s < NSUB - 1 else self.l1_chunks
            self.sub_span.append((a, b - a))

        # idx [NC][128, chunks*8] i16 (16-wrap, tiled to 128 partitions);
        # dstw [NC][128, chunks, 2] f32
        self.l1_idx = np.zeros((NC, 128, self.l1_chunks * P // 16), np.int16)
        self.l1_dstw = np.zeros((NC, P, self.l1_chunks, 2), np.float32)
        self.l1_dstw[..., 0] = 200.0
        idx_lin = np.zeros((self.l1_chunks * P,), np.int16)
        for c in range(NC):
            m = c1 == c
            idx_lin[:] = 0
            idx_lin[ech1[m] * P + ept1[m]] = srel[m].astype(np.int16)
            self.l1_dstw[c, :, :, 0] = 200.0
            self.l1_dstw[c, :, :, 1] = 0.0
            self.l1_dstw[c, ept1[m], ech1[m], 0] = loc1[m]
            self.l1_dstw[c, ept1[m], ech1[m], 1] = w1[m]
            wrapped = idx_lin.reshape(-1, 16).T  # [16, chunks*8]
            self.l1_idx[c] = np.tile(wrapped, (8, 1))

    def signature(self):
        return (
            tuple(self.l0_cap.tolist()),
            tuple(self.l1_cap.ravel().tolist()),
        )


# ----------------------------------------------------------------------------
# Program construction
# ----------------------------------------------------------------------------
def _build_program(prep, has_b0, has_b1, has_bmu, has_bvar, rs_bf16):
    nc = bacc.Bacc(num_devices=NC, name="gnn_sage_v2", num_swdge_queues=2)

    l0c = prep.l0_chunks
    l1c = prep.l1_chunks
    rs_dt = bf16 if rs_bf16 else f32

    halo_d = nc.dram_tensor("halo", (P, l0c, F_IN), bf16, kind="ExternalInput")
    xselfT_d = nc.dram_tensor("xselfT", (F_IN, R0), bf16, kind="ExternalInput")
    l0_dstw_d = nc.dram_tensor("l0_dstw", (P, l0c, 2), f32, kind="ExternalInput")
    l1_idx_d = nc.dram_tensor("l1_idx", (128, l1c * P // 16), i16, kind="ExternalInput")
    l1_dstw_d = nc.dram_tensor("l1_dstw", (P, l1c, 2), f32, kind="ExternalInput")
    ws0_d = nc.dram_tensor("W_self0", (F_IN, H), bf16, kind="ExternalInput")
    wn0_d = nc.dram_tensor("W_neigh0", (F_IN, H), bf16, kind="ExternalInput")
    ws1_d = nc.dram_tensor("W_self1", (H, H), bf16, kind="ExternalInput")
    wn1_d = nc.dram_tensor("W_neigh1", (H, H), bf16, kind="ExternalInput")
    wmu_d = nc.dram_tensor("W_mu", (H, L), bf16, kind="ExternalInput")
    wvar_d = nc.dram_tensor("W_var", (H, L), bf16, kind="ExternalInput")
    iota_d = nc.dram_tensor("iota128", (P, P), bf16, kind="ExternalInput")
    b_d = {}
    for name, sz, has in (
        ("b0", H, has_b0),
        ("b1", H, has_b1),
        ("b_mu", L, has_bmu),
        ("b_var", L, has_bvar),
    ):
        if has:
            b_d[name] = nc.dram_tensor(name, (sz,), f32, kind="ExternalInput")

    # h1 by subrange (so layer-1 gathers depend only on their slice)
    sub_rows = [SUBR, SUBR, R0 - 2 * SUBR]
    h1_d = [
        nc.dram_tensor(f"h1_{s}", (sub_rows[s], H), bf16, kind="Internal")
        for s in range(NSUB)
    ]
    h1T_d = [
        nc.dram_tensor(f"h1T_{k}", (P, B1), bf16, kind="Internal") for k in range(2)
    ]
    partials_d = [
        nc.dram_tensor(f"s1_partials_{h}", (NC * B1 // 2, H), rs_dt, kind="Internal")
        for h in range(2)
    ]
    rs_d = [
        nc.dram_tensor(f"s1_reduced_{h}", (B1 // 2, H), rs_dt, kind="Internal")
        for h in range(2)
    ]

    zloc_d = nc.dram_tensor("z_loc", (B1, L), f32, kind="ExternalOutput")
    zscale_d = nc.dram_tensor("z_scale", (B1, L), f32, kind="ExternalOutput")

    AT = mybir.ActivationFunctionType
    OP = mybir.AluOpType

    # L0 tile groups
    groups = [list(range(g, min(g + GTILES, T0))) for g in range(0, T0, GTILES)]
    # subphase end tile: last L0 tile whose rows feed h1_{s}
    sub_end_tile = [SUBR // P - 1, 2 * SUBR // P - 1, T0 - 1]  # 15, 31, 48

    # halve tiles for RS overlap: half h owns permuted tiles with
    # (tile % T1) in [h*5, h*5+5) -> output rows [c*B1 + h*640 ...)
    def tile_half(t):
        return 0 if (t % T1) < T1 // 2 else 1

    with TileContext(nc, num_cores=NC) as tc:
        with (
            tc.tile_pool(name="const", bufs=1) as cp,
            tc.tile_pool(name="stage0", bufs=2) as stagep,
            tc.tile_pool(name="stage1", bufs=1) as stage1p,
            tc.tile_pool(name="onehot", bufs=8) as mp,
            tc.tile_pool(name="small", bufs=4) as sp,
            tc.tile_pool(name="ps_seg", bufs=2, space="PSUM") as ps_seg,
            tc.tile_pool(name="ps_tr", bufs=2, space="PSUM") as ps_tr,
            tc.tile_pool(name="ps_out", bufs=3, space="PSUM") as ps_out,
        ):
            # ---- constants / resident tensors ----
            iota_sb = cp.tile([P, P], bf16)
            nc.sync.dma_start(out=iota_sb[:], in_=iota_d[:])
            ident_bf = cp.tile([P, P], bf16, tag="ident_bf", name="ident_bf")
            make_identity(nc, ident_bf[:])
            if not rs_bf16:
                ident_f32 = cp.tile([P, P], f32, tag="ident_f32", name="ident_f32")
                make_identity(nc, ident_f32[:])
            ident_rs = ident_bf if rs_bf16 else ident_f32
            ws0_sb = cp.tile([P, H], bf16)
            nc.sync.dma_start(out=ws0_sb[:], in_=ws0_d[:])
            wn0_sb = cp.tile([P, H], bf16)
            nc.sync.dma_start(out=wn0_sb[:], in_=wn0_d[:])
            ws1_sb = [cp.tile([P, H], bf16, tag=f"ws1_{k}", name=f"ws1_{k}") for k in range(2)]
            wn1_sb = [cp.tile([P, H], bf16, tag=f"wn1_{k}", name=f"wn1_{k}") for k in range(2)]
            wmu_sb = [cp.tile([P, L], bf16, tag=f"wmu_{k}", name=f"wmu_{k}") for k in range(2)]
            wvar_sb = [cp.tile([P, L], bf16, tag=f"wvar_{k}", name=f"wvar_{k}") for k in range(2)]
            for k in range(2):
                sl = slice(k * P, (k + 1) * P)
                nc.sync.dma_start(out=ws1_sb[k][:], in_=ws1_d[sl, :])
                nc.sync.dma_start(out=wn1_sb[k][:], in_=wn1_d[sl, :])
                nc.sync.dma_start(out=wmu_sb[k][:], in_=wmu_d[sl, :])
                nc.sync.dma_start(out=wvar_sb[k][:], in_=wvar_d[sl, :])
            if b_d:
                ones_sb = cp.tile([1, P], f32)
                nc.vector.memset(ones_sb[:], 1.0)
                brow = {}
                for name, hd in b_d.items():
                    t = cp.tile([1, hd.shape[0]], f32, tag=f"brow_{name}", name=f"brow_{name}")
                    nc.sync.dma_start(out=t[:], in_=hd[:].rearrange("n -> 1 n"))
                    brow[name] = t

            eps_sb = cp.tile([P, 1], f32, tag="eps", name="eps")
            nc.vector.memset(eps_sb[:], 1e-24)
            xselfT_sb = cp.tile([F_IN, R0], bf16)
            nc.sync.dma_start(out=xselfT_sb[:], in_=xselfT_d[:])
            dstw0_sb = cp.tile([P, l0c, 2], f32)
            nc.sync.dma_start(out=dstw0_sb[:], in_=l0_dstw_d[:])
            dstw1_sb = cp.tile([P, l1c, 2], f32)
            nc.sync.dma_start(out=dstw1_sb[:], in_=l1_dstw_d[:])
            idx1_sb = cp.tile([128, l1c * P // 16], i16)
            nc.sync.dma_start(out=idx1_sb[:], in_=l1_idx_d[:])

            # layer-1 staging: SBUF-resident, one tile per gather so layer-1
            # compute can start as soon as its chunks have landed
            g1tiles = {}  # (s, k) -> (tile, chunk0, nch)
            for s in range(NSUB):
                c0chunk, nchunk = prep.sub_span[s]
                for ki, subi in enumerate(range(0, nchunk, GCHUNKS)):
                    k = min(GCHUNKS, nchunk - subi)
                    g1tiles[(s, ki)] = (
                        stage1p.tile([P, k, H], bf16, tag=f"s1_{s}_{ki}", name=f"s1_{s}_{ki}"),
                        c0chunk + subi,
                        k,
                    )

            def chunk_stage(c):
                """stage AP slice for global layer-1 chunk id c."""
                for s in range(NSUB):
                    c0chunk, nchunk = prep.sub_span[s]
                    if c0chunk <= c < c0chunk + nchunk:
                        ki = (c - c0chunk) // GCHUNKS
                        tile, t0c, _ = g1tiles[(s, ki)]
                        return tile[:, c - t0c, :]
                raise AssertionError(c)

            def normalize(ps, tag):
                """relu -> L2-normalize rows of ps [P, H] -> bf16 tile."""
                hp = sp.tile([P, H], bf16, tag=f"{tag}_hp")
                nc.scalar.activation(hp[:], ps[:], AT.Relu)
                sq = sp.tile([P, H], bf16, tag=f"{tag}_sq")
                ss = sp.tile([P, 1], f32, tag=f"{tag}_ss")
                nc.scalar.activation(sq[:], hp[:], AT.Square, accum_out=ss[:])
                nrm = sp.tile([P, 1], f32, tag=f"{tag}_nrm")
                nc.scalar.activation(nrm[:], ss[:], AT.Sqrt, bias=eps_sb[:, 0:1])
                rinv = sp.tile([P, 1], f32, tag=f"{tag}_rinv")
                nc.vector.reciprocal(rinv[:], nrm[:])
                hn = sp.tile([P, H], bf16, tag=f"{tag}_hn")
                nc.vector.tensor_scalar(
                    out=hn[:], in0=hp[:], scalar1=rinv[:, 0:1], scalar2=None, op0=OP.mult
                )
                return hn

            # ================= Layer 0 =================
            # L1 gather tasks are spread over the L0 groups (<=2 per group) so
            # their DMA traffic never starves the L0 halo-slab pipeline.
            pending = []  # (s, ki) ready to issue
            launched_sub = [False] * NSUB
            gq = [0]

            def emit_gather(s, ki):
                tile, c0chunk, k = g1tiles[(s, ki)]
                nreg = nc.gpsimd.to_reg(k * P)
                nc.gpsimd.dma_gather(
                    out_ap=tile[:],
                    in_ap=h1_d[s][:],
                    idxs_ap=idx1_sb[:, c0chunk * (P // 16) : (c0chunk + k) * (P // 16)],
                    num_idxs=k * P,
                    num_idxs_reg=nreg,
                    elem_size=H,
                    queue_num=gq[0] % 2,
                )
                nc.gpsimd.free_register(nreg)
                gq[0] += 1

            for tiles in groups:
                ch_lo = int(prep.l0_chunk_start[tiles[0]])
                ch_hi = int(prep.l0_chunk_start[tiles[-1] + 1])
                sg = ch_hi - ch_lo
                stage = stagep.tile([P, sg, F_IN], bf16, tag="stage")
                nc.sync.dma_start(out=stage[:], in_=halo_d[:, ch_lo:ch_hi, :])

                for t in tiles:
                    c0 = int(prep.l0_chunk_start[t])
                    nch = int(prep.l0_cap[t])
                    ps_a = ps_seg.tile([P, P], f32, tag="ps_a", name="ps_a")
                    for j in range(nch):
                        lc = c0 - ch_lo + j
                        m = mp.tile([P, P], bf16, tag="m")
                        # split one-hot builds between DVE and the (mostly
                        # idle) gpsimd engine
                        eng = nc.gpsimd if (c0 + j) % 4 == 3 else nc.vector
                        eng.tensor_scalar(
                            out=m[:],
                            in0=iota_sb[:],
                            scalar1=dstw0_sb[:, c0 + j, 0:1],
                            scalar2=dstw0_sb[:, c0 + j, 1:2],
                            op0=OP.is_equal,
                            op1=OP.mult,
                        )
                        nc.tensor.matmul(
                            out=ps_a[:],
                            lhsT=stage[:, lc, :],
                            rhs=m[:],
                            start=(j == 0),
                            stop=(j == nch - 1),
                        )
                    aggT_sb = sp.tile([P, P], bf16, tag="aggT")
                    nc.vector.tensor_copy(out=aggT_sb[:], in_=ps_a[:])

                    ps_o = ps_out.tile([P, H], f32, tag="ps_o", name="ps_o")
                    nc.tensor.matmul(
                        out=ps_o[:],
                        lhsT=xselfT_sb[:, t * P : (t + 1) * P],
                        rhs=ws0_sb[:],
                        start=True,
                        stop=False,
                    )
                    nc.tensor.matmul(
                        out=ps_o[:], lhsT=aggT_sb[:], rhs=wn0_sb[:], start=False,
                        stop=not has_b0,
                    )
                    if has_b0:
                        nc.tensor.matmul(
                            out=ps_o[:], lhsT=ones_sb[:], rhs=brow["b0"][:],
                            start=False, stop=True,
                        )
                    h1n = normalize(ps_o, "l0")
                    s = min(t // (SUBR // P), NSUB - 1)
                    r0 = t * P - s * SUBR
                    nc.sync.dma_start(out=h1_d[s][r0 : r0 + P, :], in_=h1n[:])
                    if t < T1:
                        # also store transposed for the final self-term
                        for half in range(2):
                            hs = slice(half * P, (half + 1) * P)
                            ps_t = ps_tr.tile([P, P], bf16, tag="ps_t", name="ps_t")
                            nc.tensor.transpose(
                                out=ps_t[:], in_=h1n[:, hs], identity=ident_bf[:]
                            )
                            hT = sp.tile([P, P], bf16, tag=f"h1T_{half}")
                            nc.vector.tensor_copy(out=hT[:], in_=ps_t[:])
                            nc.sync.dma_start(
                                out=h1T_d[half][:, t * P : (t + 1) * P], in_=hT[:]
                            )

                # queue subrange gathers once their h1 rows are complete,
                # then drip-feed at most 2 per group
                for s in range(NSUB):
                    if not launched_sub[s] and tiles[-1] >= sub_end_tile[s]:
                        c0chunk, nchunk = prep.sub_span[s]
                        for ki in range(math.ceil(nchunk / GCHUNKS)):
                            pending.append((s, ki))
                        launched_sub[s] = True
                for _ in range(2):
                    if pending:
                        emit_gather(*pending.pop(0))
            while pending:
                emit_gather(*pending.pop(0))

            # ================= Layer 1 partial segment sums =================
            # process tiles half-A (output rows [0,640) per block) then half-B
            def l1_tile(t):
                ps_s = ps_out.tile([P, H], f32, tag="ps_o", name="ps_s")
                chunks = []
                for s in range(NSUB):
                    cs = int(prep.l1_chunk_start[s * T1P + t])
                    ce = int(prep.l1_chunk_start[s * T1P + t + 1]) if s * T1P + t + 1 <= NSUB * T1P else l1c
                    chunks.extend(range(cs, ce))
                for j, k in enumerate(chunks):
                    m = mp.tile([P, P], bf16, tag="m")
                    nc.vector.tensor_scalar(
                        out=m[:],
                        in0=iota_sb[:],
                        scalar1=dstw1_sb[:, k, 0:1],
                        scalar2=dstw1_sb[:, k, 1:2],
                        op0=OP.is_equal,
                        op1=OP.mult,
                    )
                    nc.tensor.matmul(
                        out=ps_s[:],
                        lhsT=m[:],
                        rhs=chunk_stage(k),
                        start=(j == 0),
                        stop=(j == len(chunks) - 1),
                    )
                s_sb = sp.tile([P, H], rs_dt, tag="s1")
                nc.vector.tensor_copy(out=s_sb[:], in_=ps_s[:])
                h = tile_half(t)
                blk = t // T1  # which core's block
                within = (t % T1) - h * (T1 // 2)
                row0 = blk * (B1 // 2) + within * P
                nc.sync.dma_start(out=partials_d[h][row0 : row0 + P, :], in_=s_sb[:])

            halves = [[t for t in range(T1P) if tile_half(t) == h] for h in range(2)]
            for t in halves[0]:
                l1_tile(t)
            nc.gpsimd.collective_compute(
                kind="ReduceScatter",
                op=mybir.AluOpType.add,
                replica_groups=[list(range(NC))],
                ins=[partials_d[0][:]],
                outs=[rs_d[0][:]],
            )
            for t in halves[1]:
                l1_tile(t)
            nc.gpsimd.collective_compute(
                kind="ReduceScatter",
                op=mybir.AluOpType.add,
                replica_groups=[list(range(NC))],
                ins=[partials_d[1][:]],
                outs=[rs_d[1][:]],
            )

            # ================= Layer 1 final + heads =================
            for t in range(T1):
                h = 0 if t < T1 // 2 else 1
                within = t - h * (T1 // 2)
                rows = slice(within * P, (within + 1) * P)
                rs_sb = sp.tile([P, H], rs_dt, tag="rs")
                nc.sync.dma_start(out=rs_sb[:], in_=rs_d[h][rows, :])

                aggT1 = []
                hdT1 = []
                for half in range(2):
                    hs = slice(half * P, (half + 1) * P)
                    ps_t = ps_tr.tile([P, P], rs_dt, tag="ps_t", name="ps_t")
                    nc.tensor.transpose(out=ps_t[:], in_=rs_sb[:, hs], identity=ident_rs[:])
                    a = sp.tile([P, P], bf16, tag=f"aggT1_{half}")
                    nc.vector.tensor_copy(out=a[:], in_=ps_t[:])
                    aggT1.append(a)
                    hT = sp.tile([P, P], bf16, tag=f"hdT1_{half}")
                    nc.sync.dma_start(
                        out=hT[:], in_=h1T_d[half][:, t * P : (t + 1) * P]
                    )
                    hdT1.append(hT)

                ps_o = ps_out.tile([P, H], f32, tag="ps_o", name="ps_o")
                nc.tensor.matmul(out=ps_o[:], lhsT=hdT1[0][:], rhs=ws1_sb[0][:], start=True, stop=False)
                nc.tensor.matmul(out=ps_o[:], lhsT=hdT1[1][:], rhs=ws1_sb[1][:], start=False, stop=False)
                nc.tensor.matmul(out=ps_o[:], lhsT=aggT1[0][:], rhs=wn1_sb[0][:], start=False, stop=False)
                nc.tensor.matmul(
                    out=ps_o[:], lhsT=aggT1[1][:], rhs=wn1_sb[1][:], start=False,
                    stop=not has_b1,
                )
                if has_b1:
                    nc.tensor.matmul(
                        out=ps_o[:], lhsT=ones_sb[:], rhs=brow["b1"][:], start=False, stop=True
                    )
                h2n = normalize(ps_o, "l1")

                h2T = []
                for half in range(2):
                    hs = slice(half * P, (half + 1) * P)
                    ps_t = ps_tr.tile([P, P], bf16, tag="ps_t", name="ps_t")
                    nc.tensor.transpose(out=ps_t[:], in_=h2n[:, hs], identity=ident_bf[:])
                    hh = sp.tile([P, P], bf16, tag=f"h2T_{half}")
                    nc.vector.tensor_copy(out=hh[:], in_=ps_t[:])
                    h2T.append(hh)

                ps_zl = ps_seg.tile([P, L], f32, tag="ps_a", name="ps_zl")
                nc.tensor.matmul(out=ps_zl[:], lhsT=h2T[0][:], rhs=wmu_sb[0][:], start=True, stop=False)
                nc.tensor.matmul(
                    out=ps_zl[:], lhsT=h2T[1][:], rhs=wmu_sb[1][:], start=False,
                    stop=not has_bmu,
                )
                if has_bmu:
                    nc.tensor.matmul(
                        out=ps_zl[:], lhsT=ones_sb[:], rhs=brow["b_mu"][:], start=False, stop=True
                    )
                zl_sb = sp.tile([P, L], f32, tag="zl")
                nc.vector.tensor_copy(out=zl_sb[:], in_=ps_zl[:])
                nc.sync.dma_start(out=zloc_d[t * P : (t + 1) * P, :], in_=zl_sb[:])

                ps_zs = ps_seg.tile([P, L], f32, tag="ps_a", name="ps_zs")
                nc.tensor.matmul(out=ps_zs[:], lhsT=h2T[0][:], rhs=wvar_sb[0][:], start=True, stop=False)
                nc.tensor.matmul(
                    out=ps_zs[:], lhsT=h2T[1][:], rhs=wvar_sb[1][:], start=False,
                    stop=not has_bvar,
                )
                if has_bvar:
                    nc.tensor.matmul(
                        out=ps_zs[:], lhsT=ones_sb[:], rhs=brow["b_var"][:], start=False, stop=True
                    )
                zs_sb = sp.tile([P, L], f32, tag="zs")
                nc.scalar.activation(zs_sb[:], ps_zs[:], AT.Exp)
                nc.vector.tensor_scalar_add(zs_sb[:], zs_sb[:], 1e-6)
                nc.sync.dma_start(out=zscale_d[t * P : (t + 1) * P, :], in_=zs_sb[:])

    nc.compile()
    return nc


# ----------------------------------------------------------------------------
# Entry point
# ----------------------------------------------------------------------------
_CACHE = {}
RS_BF16 = True


def prepare(inputs):
    """Host preprocessing + program build.  Returns (nc, in_maps, postprocess)."""
    x = np.asarray(inputs["x"], np.float32)
    prep = _Prep(x, inputs["src0"], inputs["dst0"], inputs["src1"], inputs["dst1"])

    b0 = np.asarray(inputs["b0"], np.float32)
    b1 = np.asarray(inputs["b1"], np.float32)
    bmu = np.asarray(inputs["b_mu"], np.float32)
    bvar = np.asarray(inputs["b_var"], np.float32)
    has_b0, has_b1 = bool(np.any(b0)), bool(np.any(b1))
    has_bmu, has_bvar = bool(np.any(bmu)), bool(np.any(bvar))

    key = (prep.signature(), has_b0, has_b1, has_bmu, has_bvar, RS_BF16)
    if key not in _CACHE:
        _CACHE[key] = _build_program(prep, has_b0, has_b1, has_bmu, has_bvar, RS_BF16)
    nc = _CACHE[key]

    iota = np.broadcast_to(np.arange(P, dtype=np.float32), (P, P)).astype(bfnp)
    common = {
        "W_self0": np.asarray(inputs["W_self0"], np.float32).astype(bfnp),
        "W_neigh0": np.asarray(inputs["W_neigh0"], np.float32).astype(bfnp),
        "W_self1": np.asarray(inputs["W_self1"], np.float32).astype(bfnp),
        "W_neigh1": np.asarray(inputs["W_neigh1"], np.float32).astype(bfnp),
        "W_mu": np.asarray(inputs["W_mu"], np.float32).astype(bfnp),
        "W_var": np.asarray(inputs["W_var"], np.float32).astype(bfnp),
        "iota128": iota.copy(),
    }
    if has_b0:
        common["b0"] = b0
    if has_b1:
        common["b1"] = b1
    if has_bmu:
        common["b_mu"] = bmu
    if has_bvar:
        common["b_var"] = bvar

    in_maps = []
    for c in range(NC):
        m = dict(common)
        m["halo"] = prep.halo[c]
        m["xselfT"] = prep.xselfT[c]
        m["l0_dstw"] = prep.l0_dstw[c]
        m["l1_idx"] = prep.l1_idx[c]
        m["l1_dstw"] = prep.l1_dstw[c]
        in_maps.append(m)

    def postprocess(results):
        z_loc = np.empty((N2, L), np.float32)
        z_scale = np.empty((N2, L), np.float32)
        nvalid = N2 // NC
        for c in range(NC):
            z_loc[c::NC] = results[c]["z_loc"][:nvalid]
            z_scale[c::NC] = results[c]["z_scale"][:nvalid]
        return z_loc, z_scale

    return nc, in_maps, postprocess


def kernel(**inputs):
    assert int(inputs.get("n_dst0", N1)) == N1 and int(inputs.get("n_dst1", N2)) == N2
    nc, in_maps, postprocess = prepare(inputs)
    res = run_bass_kernel_spmd(nc, in_maps, core_ids=list(range(NC)))
    return postprocess(res.results)


# revision 16
# speedup vs baseline: 1.4036x; 1.4036x over previous
"""Trainium2 Bass kernel for a 2-layer GraphSAGE(mean) encoder (8 NeuronCores).

Sharding (per spec hint: dst-node partition + per-partition halo of src feats):
  - Layer 0: core c owns dst0 nodes {d : d % 8 == c} (6250 -> 49 tiles of 128
    rows).  Host prep stages, per core, the incoming-edge halo: for every
    edge a bf16 log1p(x[src]) row placed at its (chunk, partition) slot,
    laid out partition-major so the device streams it with dense,
    full-bandwidth DMAs.  Segment-sum runs on the PE via one-hot matmuls
    (M[e,d] = (dstloc[e]==d)*inv_cnt, built on the DVE in bf16).
  - Layer 1: edges are assigned to cores by src1 % 8 so every message row is
    core-local; h1 rows are fetched with dma_gather (int16 idx) in 3
    src-subranges so the gathers overlap the tail of layer 0; each core
    computes partial segment sums over all 10000 dst1 nodes (permuted
    layout grouped by dst1 % 8) and two ReduceScatter(add) halves deliver
    each core its own 1250 dst1 rows (second half overlaps head compute).
  - Weights are replicated (bf16); final projection / relu / L2-normalize /
    heads run on the owning core; the host interleaves per-core outputs.

kernel(**inputs) takes the FULL inputs (as produced by reference.setup_inputs)
and returns (z_loc, z_scale) as float32 numpy arrays of shape [10000, 32].
"""

import math

import numpy as np
import ml_dtypes

import concourse.bass as bass
import concourse.bacc as bacc
import concourse.mybir as mybir
from concourse.bass_utils import run_bass_kernel_spmd
from concourse.masks import make_identity
from concourse.tile import TileContext

# ----------------------------------------------------------------------------
# Problem constants (hardcoded; the harness always uses these shapes).
# ----------------------------------------------------------------------------
N0, N1, N2 = 200000, 50000, 10000
E0, E1 = 800000, 160000
F_IN, H, L = 128, 256, 32
NC = 8
P = 128

T0 = math.ceil(N1 // NC / P)  # 49 layer-0 dst tiles per core
R0 = T0 * P  # 6272 padded local rows per core
GTILES = 2  # L0 tiles per staged group
B1 = math.ceil(N2 // NC / P) * P  # 1280: padded per-core block of dst1 nodes
T1P = NC * B1 // P  # 80 partial tiles (10240 permuted rows)
T1 = B1 // P  # 10 final tiles per core
NSUB = 3  # layer-1 src subranges (for gather/L0 overlap)
SUBR = 2048  # h1 rows per subrange (last one has R0-2*2048=2176)
EPS_NORM = 1e-12
GCHUNKS = 8  # max chunks (128 rows) per dma_gather instruction

f32 = mybir.dt.float32
bf16 = mybir.dt.bfloat16
i16 = mybir.dt.int16
bfnp = ml_dtypes.bfloat16


def _ranks_from_sorted(keys_sorted):
    """rank of each element within its equal-key run (keys_sorted ascending)."""
    n = keys_sorted.shape[0]
    if n == 0:
        return np.zeros(0, np.int64)
    new_run = np.empty(n, bool)
    new_run[0] = True
    new_run[1:] = keys_sorted[1:] != keys_sorted[:-1]
    starts = np.flatnonzero(new_run)
    run_ids = np.cumsum(new_run) - 1
    return np.arange(n) - starts[run_ids]


def _chunk_layout(core, cell, n_cells):
    """Shared chunk layout: cap[cell] = ceil(max_core count / P) chunks.

    Returns (cap, chunk_start, total, e_chunk, e_part) where e_chunk/e_part
    give each edge's slot (chunks shared across cores, slots per core).
    """
    counts = np.zeros((NC, n_cells), np.int64)
    np.add.at(counts, (core, cell), 1)
    cap = np.ceil(counts.max(axis=0) / P).astype(np.int64)
    chunk_start = np.concatenate([[0], np.cumsum(cap)])
    total = int(chunk_start[-1])
    order = np.lexsort((cell, core))
    key = core.astype(np.int64) * n_cells + cell
    ranks = _ranks_from_sorted(key[order])
    e_chunk = np.empty(core.shape[0], np.int64)
    e_part = np.empty(core.shape[0], np.int64)
    e_chunk[order] = chunk_start[cell[order]] + ranks // P
    e_part[order] = ranks % P
    return cap, chunk_start, total, e_chunk, e_part


class _Prep:
    """Host-side graph preprocessing shared by all cores."""

    def __init__(self, x, src0, dst0, src1, dst1):
        src0 = np.asarray(src0).astype(np.int64)
        dst0 = np.asarray(dst0).astype(np.int64)
        src1 = np.asarray(src1).astype(np.int64)
        dst1 = np.asarray(dst1).astype(np.int64)

        deg0 = np.bincount(dst0, minlength=N1)
        inv0 = (1.0 / np.maximum(deg0, 1)).astype(np.float32)
        deg1 = np.bincount(dst1, minlength=N2)
        inv1 = (1.0 / np.maximum(deg1, 1)).astype(np.float32)

        xt = np.log1p(np.asarray(x, np.float32)).astype(bfnp)  # [N0, F_IN]

        # ---------------- layer 0: dst-tile partition + edge halo ----------
        c0 = dst0 % NC
        t0 = (dst0 // NC) // P
        loc0 = ((dst0 // NC) % P).astype(np.float32)
        w0 = inv0[dst0]
        cap0, cs0, self.l0_chunks, ech, ept = _chunk_layout(c0, t0, T0)
        self.l0_cap = cap0
        self.l0_chunk_start = cs0
        # halo [NC][128, chunks, F_IN] bf16; dstw [NC][128, chunks, 2] f32
        self.halo = np.zeros((NC, P, self.l0_chunks, F_IN), bfnp)
        self.l0_dstw = np.zeros((NC, P, self.l0_chunks, 2), np.float32)
        self.l0_dstw[..., 0] = 200.0  # pad dstloc -> one-hot of zeros
        self.halo[c0, ept, ech] = xt[src0]
        self.l0_dstw[c0, ept, ech, 0] = loc0
        self.l0_dstw[c0, ept, ech, 1] = w0

        # per-core self rows, transposed: xselfT [NC][F_IN, R0] bf16
        self.xselfT = np.zeros((NC, F_IN, R0), bfnp)
        for c in range(NC):
            rows = xt[c::NC][: N1 // NC]  # [6250, F_IN]
            self.xselfT[c, :, : rows.shape[0]] = rows.T

        # ---------------- layer 1: src-core partition, permuted dst -------
        c1 = src1 % NC
        srow = src1 // NC  # local h1 row on owning core
        sub = np.minimum(srow // SUBR, NSUB - 1)
        srel = srow - sub * SUBR  # row within subrange tensor
        pd = (dst1 % NC) * B1 + dst1 // NC
        t1 = pd // P
        loc1 = (pd % P).astype(np.float32)
        w1 = inv1[dst1]
        cell = sub * T1P + t1  # layout: subphase-major, then tile
        cap1, cs1, self.l1_chunks, ech1, ept1 = _chunk_layout(c1, cell, NSUB * T1P)
        self.l1_cap = cap1.reshape(NSUB, T1P)
        self.l1_chunk_start = cs1.reshape(-1)[: NSUB * T1P + 1]
        # subphase chunk spans
        self.sub_span = []  # (chunk0, nchunks) per subphase
        for s in range(NSUB):
            a = int(cs1[s * T1P])
            b = int(cs1[(s + 1) * T1P]) if s < NSUB - 1 else self.l1_chunks
            self.sub_span.append((a, b - a))

        # idx [NC][128, chunks*8] i16 (16-wrap, tiled to 128 partitions);
        # dstw [NC][128, chunks, 2] f32
        self.l1_idx = np.zeros((NC, 128, self.l1_chunks * P // 16), np.int16)
        self.l1_dstw = np.zeros((NC, P, self.l1_chunks, 2), np.float32)
        self.l1_dstw[..., 0] = 200.0
        idx_lin = np.zeros((self.l1_chunks * P,), np.int16)
        for c in range(NC):
            m = c1 == c
            idx_lin[:] = 0
            idx_lin[ech1[m] * P + ept1[m]] = srel[m].astype(np.int16)
            self.l1_dstw[c, :, :, 0] = 200.0
            self.l1_dstw[c, :, :, 1] = 0.0
            self.l1_dstw[c, ept1[m], ech1[m], 0] = loc1[m]
            self.l1_dstw[c, ept1[m], ech1[m], 1] = w1[m]
            wrapped = idx_lin.reshape(-1, 16).T  # [16, chunks*8]
            self.l1_idx[c] = np.tile(wrapped, (8, 1))

    def signature(self):
        return (
            tuple(self.l0_cap.tolist()),
            tuple(self.l1_cap.ravel().tolist()),
        )


# ----------------------------------------------------------------------------
# Program construction
# ----------------------------------------------------------------------------
def _build_program(prep, has_b0, has_b1, has_bmu, has_bvar, rs_bf16):
    nc = bacc.Bacc(num_devices=NC, name="gnn_sage_v2", num_swdge_queues=2)

    l0c = prep.l0_chunks
    l1c = prep.l1_chunks
    rs_dt = bf16 if rs_bf16 else f32

    halo_d = nc.dram_tensor("halo", (P, l0c, F_IN), bf16, kind="ExternalInput")
    xselfT_d = nc.dram_tensor("xselfT", (F_IN, R0), bf16, kind="ExternalInput")
    l0_dstw_d = nc.dram_tensor("l0_dstw", (P, l0c, 2), f32, kind="ExternalInput")
    l1_idx_d = nc.dram_tensor("l1_idx", (128, l1c * P // 16), i16, kind="ExternalInput")
    l1_dstw_d = nc.dram_tensor("l1_dstw", (P, l1c, 2), f32, kind="ExternalInput")
    ws0_d = nc.dram_tensor("W_self0", (F_IN, H), bf16, kind="ExternalInput")
    wn0_d = nc.dram_tensor("W_neigh0", (F_IN, H), bf16, kind="ExternalInput")
    ws1_d = nc.dram_tensor("W_self1", (H, H), bf16, kind="ExternalInput")
    wn1_d = nc.dram_tensor("W_neigh1", (H, H), bf16, kind="ExternalInput")
    wmu_d = nc.dram_tensor("W_mu", (H, L), bf16, kind="ExternalInput")
    wvar_d = nc.dram_tensor("W_var", (H, L), bf16, kind="ExternalInput")
    iota_d = nc.dram_tensor("iota128", (P, P), bf16, kind="ExternalInput")
    b_d = {}
    for name, sz, has in (
        ("b0", H, has_b0),
        ("b1", H, has_b1),
        ("b_mu", L, has_bmu),
        ("b_var", L, has_bvar),
    ):
        if has:
            b_d[name] = nc.dram_tensor(name, (sz,), f32, kind="ExternalInput")

    # h1 by subrange (so layer-1 gathers depend only on their slice)
    sub_rows = [SUBR, SUBR, R0 - 2 * SUBR]
    h1_d = [
        nc.dram_tensor(f"h1_{s}", (sub_rows[s], H), bf16, kind="Internal")
        for s in range(NSUB)
    ]
    h1T_d = [
        nc.dram_tensor(f"h1T_{k}", (P, B1), bf16, kind="Internal") for k in range(2)
    ]
    partials_d = [
        nc.dram_tensor(f"s1_partials_{h}", (NC * B1 // 2, H), rs_dt, kind="Internal")
        for h in range(2)
    ]
    rs_d = [
        nc.dram_tensor(f"s1_reduced_{h}", (B1 // 2, H), rs_dt, kind="Internal")
        for h in range(2)
    ]

    zloc_d = nc.dram_tensor("z_loc", (B1, L), f32, kind="ExternalOutput")
    zscale_d = nc.dram_tensor("z_scale", (B1, L), f32, kind="ExternalOutput")

    AT = mybir.ActivationFunctionType
    OP = mybir.AluOpType

    # L0 tile groups
    groups = [list(range(g, min(g + GTILES, T0))) for g in range(0, T0, GTILES)]
    # subphase end tile: last L0 tile whose rows feed h1_{s}
    sub_end_tile = [SUBR // P - 1, 2 * SUBR // P - 1, T0 - 1]  # 15, 31, 48

    # halve tiles for RS overlap: half h owns permuted tiles with
    # (tile % T1) in [h*5, h*5+5) -> output rows [c*B1 + h*640 ...)
    def tile_half(t):
        return 0 if (t % T1) < T1 // 2 else 1

    with TileContext(nc, num_cores=NC) as tc:
        with (
            tc.tile_pool(name="const", bufs=1) as cp,
            tc.tile_pool(name="stage0", bufs=2) as stagep,
            tc.tile_pool(name="stage1", bufs=1) as stage1p,
            tc.tile_pool(name="onehot", bufs=8) as mp,
            tc.tile_pool(name="small", bufs=4) as sp,
            tc.tile_pool(name="ps_seg", bufs=2, space="PSUM") as ps_seg,
            tc.tile_pool(name="ps_tr", bufs=2, space="PSUM") as ps_tr,
            tc.tile_pool(name="ps_out", bufs=3, space="PSUM") as ps_out,
        ):
            # ---- constants / resident tensors ----
            iota_sb = cp.tile([P, P], bf16)
            nc.sync.dma_start(out=iota_sb[:], in_=iota_d[:])
            ident_bf = cp.tile([P, P], bf16, tag="ident_bf", name="ident_bf")
            make_identity(nc, ident_bf[:])
            if not rs_bf16:
                ident_f32 = cp.tile([P, P], f32, tag="ident_f32", name="ident_f32")
                make_identity(nc, ident_f32[:])
            ident_rs = ident_bf if rs_bf16 else ident_f32
            ws0_sb = cp.tile([P, H], bf16)
            nc.sync.dma_start(out=ws0_sb[:], in_=ws0_d[:])
            wn0_sb = cp.tile([P, H], bf16)
            nc.sync.dma_start(out=wn0_sb[:], in_=wn0_d[:])
            ws1_sb = [cp.tile([P, H], bf16, tag=f"ws1_{k}", name=f"ws1_{k}") for k in range(2)]
            wn1_sb = [cp.tile([P, H], bf16, tag=f"wn1_{k}", name=f"wn1_{k}") for k in range(2)]
            wmu_sb = [cp.tile([P, L], bf16, tag=f"wmu_{k}", name=f"wmu_{k}") for k in range(2)]
            wvar_sb = [cp.tile([P, L], bf16, tag=f"wvar_{k}", name=f"wvar_{k}") for k in range(2)]
            for k in range(2):
                sl = slice(k * P, (k + 1) * P)
                nc.sync.dma_start(out=ws1_sb[k][:], in_=ws1_d[sl, :])
                nc.sync.dma_start(out=wn1_sb[k][:], in_=wn1_d[sl, :])
                nc.sync.dma_start(out=wmu_sb[k][:], in_=wmu_d[sl, :])
                nc.sync.dma_start(out=wvar_sb[k][:], in_=wvar_d[sl, :])
            if b_d:
                ones_sb = cp.tile([1, P], f32)
                nc.vector.memset(ones_sb[:], 1.0)
                brow = {}
                for name, hd in b_d.items():
                    t = cp.tile([1, hd.shape[0]], f32, tag=f"brow_{name}", name=f"brow_{name}")
                    nc.sync.dma_start(out=t[:], in_=hd[:].rearrange("n -> 1 n"))
                    brow[name] = t

            eps_sb = cp.tile([P, 1], f32, tag="eps", name="eps")
            nc.vector.memset(eps_sb[:], 1e-24)
            xselfT_sb = cp.tile([F_IN, R0], bf16)
            nc.sync.dma_start(out=xselfT_sb[:], in_=xselfT_d[:])
            dstw0_sb = cp.tile([P, l0c, 2], f32)
            nc.sync.dma_start(out=dstw0_sb[:], in_=l0_dstw_d[:])
            dstw1_sb = cp.tile([P, l1c, 2], f32)
            nc.sync.dma_start(out=dstw1_sb[:], in_=l1_dstw_d[:])
            idx1_sb = cp.tile([128, l1c * P // 16], i16)
            nc.sync.dma_start(out=idx1_sb[:], in_=l1_idx_d[:])

            # layer-1 staging: SBUF-resident, one tile per gather so layer-1
            # compute can start as soon as its chunks have landed
            g1tiles = {}  # (s, k) -> (tile, chunk0, nch)
            for s in range(NSUB):
                c0chunk, nchunk = prep.sub_span[s]
                for ki, subi in enumerate(range(0, nchunk, GCHUNKS)):
                    k = min(GCHUNKS, nchunk - subi)
                    g1tiles[(s, ki)] = (
                        stage1p.tile([P, k, H], bf16, tag=f"s1_{s}_{ki}", name=f"s1_{s}_{ki}"),
                        c0chunk + subi,
                        k,
                    )

            def chunk_stage(c):
                """stage AP slice for global layer-1 chunk id c."""
                for s in range(NSUB):
                    c0chunk, nchunk = prep.sub_span[s]
                    if c0chunk <= c < c0chunk + nchunk:
                        ki = (c - c0chunk) // GCHUNKS
                        tile, t0c, _ = g1tiles[(s, ki)]
                        return tile[:, c - t0c, :]
                raise AssertionError(c)

            def normalize(ps, tag):
                """relu -> L2-normalize rows of ps [P, H] -> bf16 tile."""
                hp = sp.tile([P, H], bf16, tag=f"{tag}_hp")
                nc.scalar.activation(hp[:], ps[:], AT.Relu)
                sq = sp.tile([P, H], bf16, tag=f"{tag}_sq")
                ss = sp.tile([P, 1], f32, tag=f"{tag}_ss")
                nc.scalar.activation(sq[:], hp[:], AT.Square, accum_out=ss[:])
                nrm = sp.tile([P, 1], f32, tag=f"{tag}_nrm")
                nc.scalar.activation(nrm[:], ss[:], AT.Sqrt, bias=eps_sb[:, 0:1])
                rinv = sp.tile([P, 1], f32, tag=f"{tag}_rinv")
                nc.vector.reciprocal(rinv[:], nrm[:])
                hn = sp.tile([P, H], bf16, tag=f"{tag}_hn")
                nc.vector.tensor_scalar(
                    out=hn[:], in0=hp[:], scalar1=rinv[:, 0:1], scalar2=None, op0=OP.mult
                )
                return hn

            # ================= Layer 0 =================
            # L1 gather tasks are spread over the L0 groups (<=2 per group) so
            # their DMA traffic never starves the L0 halo-slab pipeline.
            pending = []  # (s, ki) ready to issue
            launched_sub = [False] * NSUB
            gq = [0]

            def emit_gather(s, ki):
                tile, c0chunk, k = g1tiles[(s, ki)]
                nreg = nc.gpsimd.to_reg(k * P)
                nc.gpsimd.dma_gather(
                    out_ap=tile[:],
                    in_ap=h1_d[s][:],
                    idxs_ap=idx1_sb[:, c0chunk * (P // 16) : (c0chunk + k) * (P // 16)],
                    num_idxs=k * P,
                    num_idxs_reg=nreg,
                    elem_size=H,
                    queue_num=gq[0] % 2,
                )
                nc.gpsimd.free_register(nreg)
                gq[0] += 1

            for tiles in groups:
                ch_lo = int(prep.l0_chunk_start[tiles[0]])
                ch_hi = int(prep.l0_chunk_start[tiles[-1] + 1])
                sg = ch_hi - ch_lo
                stage = stagep.tile([P, sg, F_IN], bf16, tag="stage")
                nc.sync.dma_start(out=stage[:], in_=halo_d[:, ch_lo:ch_hi, :])

                for t in tiles:
                    c0 = int(prep.l0_chunk_start[t])
                    nch = int(prep.l0_cap[t])
                    ps_a = ps_seg.tile([P, P], f32, tag="ps_a", name="ps_a")
                    for j in range(nch):
                        lc = c0 - ch_lo + j
                        m = mp.tile([P, P], bf16, tag="m")
                        nc.vector.tensor_scalar(
                            out=m[:],
                            in0=iota_sb[:],
                            scalar1=dstw0_sb[:, c0 + j, 0:1],
                            scalar2=dstw0_sb[:, c0 + j, 1:2],
                            op0=OP.is_equal,
                            op1=OP.mult,
                        )
                        nc.tensor.matmul(
                            out=ps_a[:],
                            lhsT=stage[:, lc, :],
                            rhs=m[:],
                            start=(j == 0),
                            stop=(j == nch - 1),
                        )
                    aggT_sb = sp.tile([P, P], bf16, tag="aggT")
                    nc.vector.tensor_copy(out=aggT_sb[:], in_=ps_a[:])

                    ps_o = ps_out.tile([P, H], f32, tag="ps_o", name="ps_o")
                    nc.tensor.matmul(
                        out=ps_o[:],
                        lhsT=xselfT_sb[:, t * P : (t + 1) * P],
                        rhs=ws0_sb[:],
                        start=True,
                        stop=False,
                    )
                    nc.tensor.matmul(
                        out=ps_o[:], lhsT=aggT_sb[:], rhs=wn0_sb[:], start=False,
                        stop=not has_b0,
                    )
                    if has_b0:
                        nc.tensor.matmul(
                            out=ps_o[:], lhsT=ones_sb[:], rhs=brow["b0"][:],
                            start=False, stop=True,
                        )
                    h1n = normalize(ps_o, "l0")
                    s = min(t // (SUBR // P), NSUB - 1)
                    r0 = t * P - s * SUBR
                    nc.sync.dma_start(out=h1_d[s][r0 : r0 + P, :], in_=h1n[:])
                    if t < T1:
                        # also store transposed for the final self-term
                        for half in range(2):
                            hs = slice(half * P, (half + 1) * P)
                            ps_t = ps_tr.tile([P, P], bf16, tag="ps_t", name="ps_t")
                            nc.tensor.transpose(
                                out=ps_t[:], in_=h1n[:, hs], identity=ident_bf[:]
                            )
                            hT = sp.tile([P, P], bf16, tag=f"h1T_{half}")
                            nc.vector.tensor_copy(out=hT[:], in_=ps_t[:])
                            nc.sync.dma_start(
                                out=h1T_d[half][:, t * P : (t + 1) * P], in_=hT[:]
                            )

                # queue subrange gathers once their h1 rows are complete,
                # then drip-feed at most 2 per group
                for s in range(NSUB):
                    if not launched_sub[s] and tiles[-1] >= sub_end_tile[s]:
                        c0chunk, nchunk = prep.sub_span[s]
                        for ki in range(math.ceil(nchunk / GCHUNKS)):
                            pending.append((s, ki))
                        launched_sub[s] = True
                for _ in range(2):
                    if pending:
                        emit_gather(*pending.pop(0))
            while pending:
                emit_gather(*pending.pop(0))

            # ================= Layer 1 partial segment sums =================
            # process tiles half-A (output rows [0,640) per block) then half-B
            def l1_tile(t):
                ps_s = ps_out.tile([P, H], f32, tag="ps_o", name="ps_s")
                chunks = []
                for s in range(NSUB):
                    cs = int(prep.l1_chunk_start[s * T1P + t])
                    ce = int(prep.l1_chunk_start[s * T1P + t + 1]) if s * T1P + t + 1 <= NSUB * T1P else l1c
                    chunks.extend(range(cs, ce))
                for j, k in enumerate(chunks):
                    m = mp.tile([P, P], bf16, tag="m")
                    nc.vector.tensor_scalar(
                        out=m[:],
                        in0=iota_sb[:],
                        scalar1=dstw1_sb[:, k, 0:1],
                        scalar2=dstw1_sb[:, k, 1:2],
                        op0=OP.is_equal,
                        op1=OP.mult,
                    )
                    nc.tensor.matmul(
                        out=ps_s[:],
                        lhsT=m[:],
                        rhs=chunk_stage(k),
                        start=(j == 0),
                        stop=(j == len(chunks) - 1),
                    )
                s_sb = sp.tile([P, H], rs_dt, tag="s1")
                nc.vector.tensor_copy(out=s_sb[:], in_=ps_s[:])
                h = tile_half(t)
                blk = t // T1  # which core's block
                within = (t % T1) - h * (T1 // 2)
                row0 = blk * (B1 // 2) + within * P
                nc.sync.dma_start(out=partials_d[h][row0 : row0 + P, :], in_=s_sb[:])

            halves = [[t for t in range(T1P) if tile_half(t) == h] for h in range(2)]
            for t in halves[0]:
                l1_tile(t)
            nc.gpsimd.collective_compute(
                kind="ReduceScatter",
                op=mybir.AluOpType.add,
                replica_groups=[list(range(NC))],
                ins=[partials_d[0][:]],
                outs=[rs_d[0][:]],
            )
            for t in halves[1]:
                l1_tile(t)
            nc.gpsimd.collective_compute(
                kind="ReduceScatter",
                op=mybir.AluOpType.add,
                replica_groups=[list(range(NC))],
                ins=[partials_d[1][:]],
                outs=[rs_d[1][:]],
            )

            # ================= Layer 1 final + heads =================
            for t in range(T1):
                h = 0 if t < T1 // 2 else 1
                within = t - h * (T1 // 2)
                rows = slice(within * P, (within + 1) * P)
                rs_sb = sp.tile([P, H], rs_dt, tag="rs")
                nc.sync.dma_start(out=rs_sb[:], in_=rs_d[h][rows, :])

                aggT1 = []
                hdT1 = []
                for half in range(2):
                    hs = slice(half * P, (half + 1) * P)
                    ps_t = ps_tr.tile([P, P], rs_dt, tag="ps_t", name="ps_t")
                    nc.tensor.transpose(out=ps_t[:], in_=rs_sb[:, hs], identity=ident_rs[:])
                    a = sp.tile([P, P], bf16, tag=f"aggT1_{half}")
                    nc.vector.tensor_copy(out=a[:], in_=ps_t[:])
                    aggT1.append(a)
                    hT = sp.tile([P, P], bf16, tag=f"hdT1_{half}")
                    nc.sync.dma_start(
                        out=hT[:], in_=h1T_d[half][:, t * P : (t + 1) * P]
                    )
                    hdT1.append(hT)

                ps_o = ps_out.tile([P, H], f32, tag="ps_o", name="ps_o")
                nc.tensor.matmul(out=ps_o[:], lhsT=hdT1[0][:], rhs=ws1_sb[0][:], start=True, stop=False)
                nc.tensor.matmul(out=ps_o[:], lhsT=hdT1[1][:], rhs=ws1_sb[1][:], start=False, stop=False)
                nc.tensor.matmul(out=ps_o[:], lhsT=aggT1[0][:], rhs=wn1_sb[0][:], start=False, stop=False)
                nc.tensor.matmul(
                    out=ps_o[:], lhsT=aggT1[1][:], rhs=wn1_sb[1][:], start=False,
                    stop=not has_b1,
                )
                if has_b1:
                    nc.tensor.matmul(
                        out=ps_o[:], lhsT=ones_sb[:], rhs=brow["b1"][:], start=False, stop=True
                    )
                h2n = normalize(ps_o, "l1")

                h2T = []
                for half in range(2):
                    hs = slice(half * P, (half + 1) * P)
                    ps_t = ps_tr.tile([P, P], bf16, tag="ps_t", name="ps_t")
                    nc.tensor.transpose(out=ps_t[:], in_=h2n[:, hs], identity=ident_bf[:])
                    hh = sp.tile([P, P], bf16, tag=f"h2T_{half}")
                    nc.vector.tensor_copy(out=hh[:], in_=ps_t[:])
                    h2T.append(hh)

                ps_zl = ps_seg.tile([P, L], f32, tag="ps_a", name="ps_zl")
                nc.tensor.matmul(out=ps_zl[:], lhsT=h2T[0][:], rhs=wmu_sb[0][:], start=True, stop=False)
                nc.tensor.matmul(
                    out=ps_zl[:], lhsT=h2T[1][:], rhs=wmu_sb[1][:], start=False,
                    stop=not has_bmu,
                )
                if has_bmu:
                    nc.tensor.matmul(
                        out=ps_zl[:], lhsT=ones_sb[:], rhs=brow["b_mu"][:], start=False, stop=True
                    )
                zl_sb = sp.tile([P, L], f32, tag="zl")
                nc.vector.tensor_copy(out=zl_sb[:], in_=ps_zl[:])
                nc.sync.dma_start(out=zloc_d[t * P : (t + 1) * P, :], in_=zl_sb[:])

                ps_zs = ps_seg.tile([P, L], f32, tag="ps_a", name="ps_zs")
                nc.tensor.matmul(out=ps_zs[:], lhsT=h2T[0][:], rhs=wvar_sb[0][:], start=True, stop=False)
                nc.tensor.matmul(
                    out=ps_zs[:], lhsT=h2T[1][:], rhs=wvar_sb[1][:], start=False,
                    stop=not has_bvar,
                )
                if has_bvar:
                    nc.tensor.matmul(
                        out=ps_zs[:], lhsT=ones_sb[:], rhs=brow["b_var"][:], start=False, stop=True
                    )
                zs_sb = sp.tile([P, L], f32, tag="zs")
                nc.scalar.activation(zs_sb[:], ps_zs[:], AT.Exp)
                nc.vector.tensor_scalar_add(zs_sb[:], zs_sb[:], 1e-6)
                nc.sync.dma_start(out=zscale_d[t * P : (t + 1) * P, :], in_=zs_sb[:])

    nc.compile()
    return nc


# ----------------------------------------------------------------------------
# Entry point
# ----------------------------------------------------------------------------
_CACHE = {}
RS_BF16 = True


def prepare(inputs):
    """Host preprocessing + program build.  Returns (nc, in_maps, postprocess)."""
    x = np.asarray(inputs["x"], np.float32)
    prep = _Prep(x, inputs["src0"], inputs["dst0"], inputs["src1"], inputs["dst1"])

    b0 = np.asarray(inputs["b0"], np.float32)
    b1 = np.asarray(inputs["b1"], np.float32)
    bmu = np.asarray(inputs["b_mu"], np.float32)
    bvar = np.asarray(inputs["b_var"], np.float32)
    has_b0, has_b1 = bool(np.any(b0)), bool(np.any(b1))
    has_bmu, has_bvar = bool(np.any(bmu)), bool(np.any(bvar))

    key = (prep.signature(), has_b0, has_b1, has_bmu, has_bvar, RS_BF16)
    if key not in _CACHE:
        _CACHE[key] = _build_program(prep, has_b0, has_b1, has_bmu, has_bvar, RS_BF16)
    nc = _CACHE[key]

    iota = np.broadcast_to(np.arange(P, dtype=np.float32), (P, P)).astype(bfnp)
    common = {
        "W_self0": np.asarray(inputs["W_self0"], np.float32).astype(bfnp),
        "W_neigh0": np.asarray(inputs["W_neigh0"], np.float32).astype(bfnp),
        "W_self1": np.asarray(inputs["W_self1"], np.float32).astype(bfnp),
        "W_neigh1": np.asarray(inputs["W_neigh1"], np.float32).astype(bfnp),
        "W_mu": np.asarray(inputs["W_mu"], np.float32).astype(bfnp),
        "W_var": np.asarray(inputs["W_var"], np.float32).astype(bfnp),
        "iota128": iota.copy(),
    }
    if has_b0:
        common["b0"] = b0
    if has_b1:
        common["b1"] = b1
    if has_bmu:
        common["b_mu"] = bmu
    if has_bvar:
        common["b_var"] = bvar

    in_maps = []
    for c in range(NC):
        m = dict(common)
        m["halo"] = prep.halo[c]
        m["xselfT"] = prep.xselfT[c]
        m["l0_dstw"] = prep.l0_dstw[c]
        m["l1_idx"] = prep.l1_idx[c]
        m["l1_dstw"] = prep.l1_dstw[c]
        in_maps.append(m)

    def postprocess(results):
        z_loc = np.empty((N2, L), np.float32)
        z_scale = np.empty((N2, L), np.float32)
        nvalid = N2 // NC
        for c in range(NC):
            z_loc[c::NC] = results[c]["z_loc"][:nvalid]
            z_scale[c::NC] = results[c]["z_scale"][:nvalid]
        return z_loc, z_scale

    return nc, in_maps, postprocess


def kernel(**inputs):
    assert int(inputs.get("n_dst0", N1)) == N1 and int(inputs.get("n_dst1", N2)) == N2
    nc, in_maps, postprocess = prepare(inputs)
    res = run_bass_kernel_spmd(nc, in_maps, core_ids=list(range(NC)))
    return postprocess(res.results)
